# revision 18
# baseline (speedup 1.0000x reference)
"""Trainium2 Bass kernel: BoxSeg DynamicMaskHead compute_pairwise_term.

For each instance n and each of the 8 non-center taps (dy, dx) of a 3x3
dilation-2 stencil:

    out[n, t, h, w] = sp(x[h,w]) + sp(x[h+dy,w+dx]) - sp(x[h,w] + x[h+dy,w+dx])

with sp = softplus (identical to the reference's
-logaddexp(log_fg + log_fg_tap, log_bg + log_bg_tap)), and 0 where the tap
falls outside the image.  No Softplus activation table exists in this build,
so sp is computed as  E = exp(x),  L = ln(E + 1)  and the tap term as
ln(1 + E_c * E_y); E/L/ln all live in the single natural_log_exp_and_others
activation table (one table load total -- the default inserter would thrash
a 1.28us load on every Exp<->Ln transition).

Mirror symmetry  out[(dy,dx)][h,w] == out[(-dy,-dx)][h+dy,w+dx]  means only 4
of the 8 tap fields are computed; each is DMA'd to two output locations.

Layout: one partition holds a ROW-PAIR (rows 2p, 2p+1) of one whole 256x256
image.  Row shifts (dy=-2) become a partition shift materialized by a second
DMA-loaded copy (k=1); column shifts are free-dim AP offsets.  Engine split:
products on GpSimd, combines on Vector, transcendentals on Scalar, all DMAs
on the sync HWDGE ring (the SP engine does nothing else, so ring
backpressure never stalls compute; one ring sustains full DMA bandwidth).
DMA partition ranges are kept at counts with a large divisor <= 16: the
HWDGE spreads a DMA over `largest divisor of partition_count <= 16` SDMA
engines (128 -> 16 engines, 127 -> ONE engine).

The kernel is HBM-write-bandwidth bound: 16.8 MB of output per core at the
~245 GB/s per-core write ceiling.

Out-of-bounds strips are never written: run_bass_kernel_spmd pre-zeros
ExternalOutput buffers (np.zeros native / donated zero buffers under PJRT).

Sharding: data-parallel over N=64 -> 8 instances per core on 8 NeuronCores.
Self-contained: shapes hardcoded.
"""

import os

import numpy as np

N_CORES = 8
N_FULL = 64
N_PER = N_FULL // N_CORES  # 8 instances per core
H = W = 256

# Tap order matches F.unfold row-major (i, j) with center removed.
TAPS = [(-2, -2), (-2, 0), (-2, 2), (0, -2), (0, 2), (2, -2), (2, 0), (2, 2)]
NT = len(TAPS)
# Computed quarters: (tap_idx, dy, dx, mirror_tap_idx).  dy in {-2, 0}.
# Pair A = quarters 0,1 (both read the k=1 row-shifted copy, col offsets
# 0/2); pair B = quarters 2,3 (k=0 col 4 / k=1 col 4).
QUARTERS = [
    (0, -2, -2, 7),  # q0
    (1, -2, 0, 6),   # q1 (full width)
    (4, 0, 2, 3),    # q2
    (2, -2, 2, 5),   # q3
]

_CACHE = {}


def _force_combined_act_table():
    """Restrict the table-load inserter's view to the one set containing both
    Exp and Ln (others emptied, positions preserved so act_func_set_id still
    indexes the real act_info.json)."""
    import concourse.bacc as bacc
    import concourse.hw_specs as hw_specs
    import concourse.mybir as mybir

    real = dict(hw_specs.get_activation_tables("gen3"))
    target = None
    for name, fns in real.items():
        if (
            mybir.ActivationFunctionType.Exp in fns
            and mybir.ActivationFunctionType.Ln in fns
        ):
            target = name
            break
    assert target is not None, "no act table set with both Exp and Ln"
    patched = {name: (fns if name == target else set()) for name, fns in real.items()}
    bacc.get_activation_tables = lambda arch: patched
    hw_specs.get_activation_tables = lambda arch: patched


def _build_program():
    import concourse.bacc as bacc
    import concourse.mybir as mybir
    from concourse import tile

    if not os.environ.get("KERNEL_NO_ACT_PATCH"):
        _force_combined_act_table()

    f32 = mybir.dt.float32
    EXP = mybir.ActivationFunctionType.Exp
    LN = mybir.ActivationFunctionType.Ln

    def mk(base, dims, off=0):
        """Keep base's partition dim (ap[0]), replace free dims with `dims`
        ([step, count] in elements), advance offset by `off` elements."""
        c = base.copy()
        c.ap = mybir.VecI64Pair([list(c.ap[0])] + [list(d) for d in dims])
        c.offset = c.offset + off
        return c

    def mkd(base, dims, off=0):
        """Same for DRAM APs (no partition dim)."""
        c = base.copy()
        c.ap = mybir.VecI64Pair([list(d) for d in dims])
        c.offset = c.offset + off
        return c

    nc = bacc.Bacc(
        "TRN2",
        target_bir_lowering=False,
        debug=False,
        enable_asserts=False,
        num_devices=N_CORES,
    )
    x = nc.dram_tensor("x", [N_PER, H, W], f32, kind="ExternalInput").ap()
    out = nc.dram_tensor("out", [N_PER, NT, H, W], f32, kind="ExternalOutput").ap()

    XN = H * W                             # x[n] stride
    ON, OT, OR = NT * H * W, H * W, W      # out strides

    # X/E/L tile free layout per partition p (image rows 2p, 2p+1):
    #   (k, a, c): off = k*520 + a*260 + c;  c-2 = image col
    #   k=0: row 2p+a;  k=1: row 2p+a-2 (partition-shifted copy)
    def xo(k, a=0, c=0):
        return k * 520 + a * 260 + c

    # P/ln/D/o tiles: (q, a, w): off = q*512 + a*256 + w
    with tile.TileContext(nc) as tc:
        with (
            tc.tile_pool(name="io", bufs=4) as iop,
            tc.tile_pool(name="wk", bufs=4) as wp,
        ):
            for n in range(N_PER):
                X = iop.tile([128, 1040], f32, tag="X")
                # zero 2-col halos at both edges of every (k, a) row window
                nc.vector.memset(mk(X[:, 0:1], [[260, 4], [258, 2], [1, 2]]), 0.0)
                # k=0: whole image, row-pairs (reads on the scalar ring so
                # the sync ring carries only the write stream)
                nc.scalar.dma_start(
                    out=mk(X[:, 0:1], [[260, 2], [1, 256]], xo(0, 0, 2)),
                    in_=mkd(x[0, 0:2, :], [[512, 128], [256, 2], [1, 256]], n * XN),
                )
                # k=1[p] == k=0[p-1] in row-pair layout: materialize via
                # SBUF->SBUF partition-shift DMA (no HBM read; fabric-rate).
                # Split 127 partitions as 64+63 for SDMA-engine spread.
                nc.scalar.dma_start(
                    out=mk(X[1:65, 0:1], [[1, 520]], xo(1, 0, 0)),
                    in_=mk(X[0:64, 0:1], [[1, 520]], xo(0, 0, 0)),
                )
                nc.scalar.dma_start(
                    out=mk(X[65:128, 0:1], [[1, 520]], xo(1, 0, 0)),
                    in_=mk(X[64:127, 0:1], [[1, 520]], xo(0, 0, 0)),
                )
                # k=1 partition 0 has no source partition: zero it (feeds
                # only discarded outputs; zero keeps everything finite)
                nc.gpsimd.memset(mk(X[0:1, 0:1], [[1, 520]], xo(1, 0, 0)), 0.0)

                E = iop.tile([128, 1040], f32, tag="E")
                nc.scalar.activation(E[:, :], X[:, :], EXP)
                L = iop.tile([128, 1040], f32, tag="L")
                nc.scalar.activation(L[:, :], E[:, :], LN, bias=1.0)

                # P[q] = E_c * E_y  (pair A: q0,q1; pair B: q2,q3) on GpSimd
                P = wp.tile([128, 2048], f32, tag="P")
                nc.gpsimd.tensor_mul(
                    out=P[:, 0:1024],
                    in0=mk(E[:, 0:1], [[0, 2], [260, 2], [1, 256]], xo(0, 0, 2)),
                    in1=mk(E[:, 0:1], [[2, 2], [260, 2], [1, 256]], xo(1, 0, 0)),
                )
                nc.gpsimd.tensor_mul(
                    out=P[:, 1024:2048],
                    in0=mk(E[:, 0:1], [[0, 2], [260, 2], [1, 256]], xo(0, 0, 2)),
                    in1=mk(E[:, 0:1], [[520, 2], [260, 2], [1, 256]], xo(0, 0, 4)),
                )

                # ln_t = ln(P + 1), split per pair so pair A's combine can
                # start while pair B is still multiplying
                ln_t = wp.tile([128, 2048], f32, tag="ln")
                nc.scalar.activation(ln_t[:, 0:1024], P[:, 0:1024], LN, bias=1.0)
                nc.scalar.activation(ln_t[:, 1024:2048], P[:, 1024:2048], LN, bias=1.0)

                # D = ln_t - L_y (in place); o = L_c - D   (Vector, plain
                # contiguous APs per (quarter, row-of-pair) to avoid DVE
                # region-walk overhead)
                # (q, k_src, c_src) for the L_y operand of each quarter
                LY = [(0, 1, 0), (1, 1, 2), (2, 0, 4), (3, 1, 4)]
                for q, k_src, c_src in LY:
                    for a in range(2):
                        off = q * 512 + a * 256
                        nc.vector.tensor_sub(
                            out=ln_t[:, off : off + 256],
                            in0=ln_t[:, off : off + 256],
                            in1=mk(L[:, 0:1], [[1, 256]], xo(k_src, a, c_src)),
                        )
                o = wp.tile([128, 2048], f32, tag="o")
                for q in range(4):
                    for a in range(2):
                        off = q * 512 + a * 256
                        nc.vector.tensor_sub(
                            out=o[:, off : off + 256],
                            in0=mk(L[:, 0:1], [[1, 256]], xo(0, a, 2)),
                            in1=ln_t[:, off : off + 256],
                        )
                # zero the invalid column strips inside o so the DIRECT tap
                # writes can be full-width (2KB-contiguous row-pair runs):
                # q0 (dx=-2): cols {0,1};  q2/q3 (dx=+2): cols {254,255}
                nc.vector.memset(mk(o[:, 0:1], [[256, 2], [1, 2]], 0 * 512), 0.0)
                nc.vector.memset(mk(o[:, 0:1], [[256, 2], [1, 2]], 2 * 512 + 254), 0.0)
                nc.vector.memset(mk(o[:, 0:1], [[256, 2], [1, 2]], 3 * 512 + 254), 0.0)

                # stores: each quarter -> direct tap t and mirror tap tm.
                for qi, (t_idx, dy, dx, tm_idx) in enumerate(QUARTERS):
                    c0 = max(0, -dx)
                    ncols = 256 - abs(dx)
                    c0m = max(0, dx)
                    p0 = -dy // 2
                    # partition segments whose counts split across many SDMA
                    # engines (avoid 127 -> 1 engine)
                    segs = [(0, 128)] if p0 == 0 else [(1, 113), (113, 128)]
                    po = qi * 512
                    for (pa, pb) in segs:
                        npp = pb - pa
                        rom = 2 * pa + dy  # mirror-write first row
                        # direct write: always full-width (strips are zero)
                        s1 = mk(o[pa:pb, 0:1], [[1, 512]], po)
                        d1 = mkd(out[0, 0, 0:2, :], [[512, npp], [1, 512]],
                                 n * ON + t_idx * OT + (2 * pa) * OR)
                        nc.sync.dma_start(out=d1, in_=s1)
                        # mirror write: clipped columns (source range for a
                        # full-width mirror write doesn't exist)
                        if dx == 0:
                            s2 = s1
                            d2 = mkd(out[0, 0, 0:2, :], [[512, npp], [1, 512]],
                                     n * ON + tm_idx * OT + rom * OR)
                        else:
                            s2 = mk(o[pa:pb, 0:1], [[256, 2], [1, ncols]],
                                    po + c0m - dx)
                            d2 = mkd(out[0, 0, 0:2, :],
                                     [[512, npp], [256, 2], [1, ncols]],
                                     n * ON + tm_idx * OT + rom * OR + c0m)
                        nc.sync.dma_start(out=d2, in_=s2)
    nc.compile()
    return nc


def _get_program():
    if "nc" not in _CACHE:
        _CACHE["nc"] = _build_program()
    return _CACHE["nc"]


def kernel(mask_logits, pairwise_size=3, pairwise_dilation=2, **_unused):
    assert int(pairwise_size) == 3 and int(pairwise_dilation) == 2
    from concourse.bass_utils import run_bass_kernel_spmd

    xf = np.ascontiguousarray(
        np.asarray(mask_logits, dtype=np.float32).reshape(N_FULL, H, W)
    )
    nc = _get_program()
    in_maps = [
        {"x": np.ascontiguousarray(xf[c * N_PER : (c + 1) * N_PER])}
        for c in range(N_CORES)
    ]
    res = run_bass_kernel_spmd(nc, in_maps, core_ids=list(range(N_CORES)))
    return np.concatenate([res.results[c]["out"] for c in range(N_CORES)], axis=0)
